# revision 1
# baseline (speedup 1.0000x reference)
"""Discriminative-loss kernel for Trainium2 (Bass/Tile), 8-core data-parallel.

Layout / algorithm (per core = one batch sample, SPMD over 8 cores):
  inputs per core:  x [d=16, N=262144] fp32 (natural d-major layout),
                    label-derived index tensors (host re-layouts only).
  pass 1:  x --SWDGE-cast--> X_bf bf16 SBUF [128=(16j+dd), M=32768]
           xbar-transpose 128-col blocks -> X_T [n-on-partition, (c,j,17)]
           (col 16 pre-set to ones), then 2048 small accumulating matmuls
           stationary=onehot[128,8] x moving=X_T[128,17] -> PSUM [8,(16+1)]
           = per-cluster sums | counts, 4 PSUM column groups (col-tiled).
  centers: tiny ops: combine groups, divide by counts, transpose,
           replicate -> c_table [128=(16j+dd), 8] fp32.
  pass 2:  ap_gather (GPSIMD) own-center per point -> c_own fp32 ->
           ACT cast bf16 -> DVE diff/square -> matmuls vs blockdiag-ones
           reduce over dd -> s = ||x-c||^2 PSUM [4 slabs x 512] ->
           ACT sqrt -> repack DMA -> e_dense [128, 2048] (chunk-per-core) ->
           relu(d-1) -> square+mask-accumulate per cluster -> V [128, 8].
  host:    centers/dist/reg terms + final mean from [8,17] sums|counts and
           V partials (O(K^2 d) flops on reduced stats only).
"""

import contextlib
import ctypes
import sys
import types

import numpy as np

# ---------------------------------------------------------------------------
# problem constants (hardcoded per contract)
B, D, HH, WW, K = 8, 16, 512, 512, 8
N = HH * WW            # 262144 points per sample
J = 8                  # chunks per core (ap_gather core granularity: 16 parts)
NCORES = 8
DELTA_VAR = 1.0
DELTA_DIST = 2.0

_BF16 = None  # ml_dtypes.bfloat16, resolved lazily


def _bf16():
    global _BF16
    if _BF16 is None:
        import ml_dtypes

        _BF16 = np.dtype(ml_dtypes.bfloat16)
    return _BF16


# ---------------------------------------------------------------------------
# walrus workaround: this toolchain allows only ONE sync-wait per
# instruction; spread extras onto preceding same-engine nops.
def _split_multi_waits(nc):
    from concourse import mybir

    n = 0
    for f in nc.m.functions:
        for bb in f.blocks:
            new_insts = []
            for ins in bb.instructions:
                si = getattr(ins, "sync_info", None)
                waits = list(si.on_wait) if si is not None and si.on_wait else []
                if len(waits) > 1:
                    for w in waits[:-1]:
                        n += 1
                        new_insts.append(
                            mybir.InstNoOp(
                                name=f"I-waitsplit-{n}",
                                engine=ins.engine,
                                bass_nofuse=True,
                                sync_info=mybir.SyncInfo(on_wait=[w], on_update=[]),
                            )
                        )
                    si.on_wait = waits[-1:]
                new_insts.append(ins)
            bb.instructions[:] = new_insts


# ---------------------------------------------------------------------------
# NTFF profiling hook (axon): lets run_bass_kernel_spmd(trace=True) work in
# this container. Harmless if the .so lacks the symbols.
def install_ntff_hook():
    try:
        import antenv

        if "antenv.axon_hooks" in sys.modules:
            return
        so_path = "/opt/axon/libaxon_pjrt.so"
        lib = ctypes.CDLL(so_path)
        if not hasattr(lib, "axon_start_nrt_profile"):
            return
        lib.axon_start_nrt_profile.argtypes = [
            ctypes.POINTER(ctypes.c_int64),
            ctypes.c_size_t,
        ]
        lib.axon_start_nrt_profile.restype = ctypes.c_int64
        lib.axon_stop_nrt_profile.argtypes = [ctypes.c_char_p]
        lib.axon_stop_nrt_profile.restype = ctypes.c_int64

        @contextlib.contextmanager
        def _hook(output_dir, device_ids):
            import jax

            jax.devices()
            if device_ids:
                ids = (ctypes.c_int64 * len(device_ids))(*device_ids)
                rc = lib.axon_start_nrt_profile(ids, len(device_ids))
            else:
                rc = lib.axon_start_nrt_profile(None, 0)
            if rc != 0:
                raise RuntimeError(f"axon_start_nrt_profile rc={rc}")
            try:
                yield
            finally:
                n = lib.axon_stop_nrt_profile(str(output_dir).encode())
                print(f"ntff profile: {n} file(s) -> {output_dir}", file=sys.stderr)

        mod = types.ModuleType("antenv.axon_hooks")
        mod.get_axon_ntff_profile_hook = lambda: _hook
        mod.set_axon_ntff_profile_hook = lambda h: None
        sys.modules["antenv.axon_hooks"] = mod
        antenv.axon_hooks = mod
    except Exception:
        pass


# ---------------------------------------------------------------------------
def build_nc(nt=16, num_devices=NCORES):
    """Build the Bass program.  nt = number of 2048-wide column tiles of the
    per-core X_bf layout (16 for the full problem; smaller for simulation).

    Per-core point count = 8 chunks * M where M = 2048*nt.
    """
    import concourse.bass as bass
    import concourse.tile as tile
    from concourse import mybir

    assert 1 <= nt <= 16
    M = 2048 * nt          # points per chunk
    NPTS = J * M           # points per core
    CB = M // 128          # number of 128-col transpose blocks per chunk-layout
    E_COLS = 2048          # e_dense free size; rows used: 16*j + t (t < nt)

    fp32 = mybir.dt.float32
    bf16 = mybir.dt.bfloat16

    nc = bass.Bass(
        "TRN2", target_bir_lowering=False, debug=False, num_devices=num_devices
    )

    x_in = nc.dram_tensor("x", [D, NPTS], fp32, kind="ExternalInput").ap()
    oh_t = nc.dram_tensor("oh_t", [128, CB, J, K], bf16, kind="ExternalInput").ap()
    # onehot in (j,k)-partition layout: row 8*j + k, col f -> labels[j*M+f]==k
    oh_jk = nc.dram_tensor("oh_jk", [64, M], bf16, kind="ExternalInput").ap()
    lab_e = nc.dram_tensor("lab_e", [128, E_COLS], bf16, kind="ExternalInput").ap()
    # [128, 32] j-selection stationary; cols 8..32 are zero so every matmul
    # writes all 32 partitions of its column group (no stale-PSUM garbage).
    red8 = nc.dram_tensor("red8", [128, 32], bf16, kind="ExternalInput").ap()
    ones128 = nc.dram_tensor("ones128", [128, 1], fp32, kind="ExternalInput").ap()
    out_stats = nc.dram_tensor(
        "out_stats", [K, D + 1], fp32, kind="ExternalOutput"
    ).ap()
    out_var = nc.dram_tensor("out_var", [128, K], fp32, kind="ExternalOutput").ap()

    with tile.TileContext(nc) as tc, contextlib.ExitStack() as ctx:
        # ---------------- pools
        # persistent big tensors
        xbf_pool = ctx.enter_context(tc.tile_pool(name="xbf", bufs=nt))
        xt_pool = ctx.enter_context(tc.tile_pool(name="xt", bufs=min(4, nt)))
        oht_pool = ctx.enter_context(tc.tile_pool(name="oht", bufs=min(4, nt)))
        singles = ctx.enter_context(tc.tile_pool(name="singles", bufs=1))
        tiny = ctx.enter_context(tc.tile_pool(name="tiny", bufs=1))
        p2 = ctx.enter_context(tc.tile_pool(name="p2", bufs=2))
        p2b = ctx.enter_context(tc.tile_pool(name="p2b", bufs=2))
        ps_sums_pool = ctx.enter_context(
            tc.tile_pool(name="ps_sums", bufs=1, space="PSUM")
        )
        ps_e_pool = ctx.enter_context(tc.tile_pool(name="ps_e", bufs=2, space="PSUM"))
        ps_c_pool = ctx.enter_context(tc.tile_pool(name="ps_c", bufs=3, space="PSUM"))

        # ---------------- load constants
        red8_sb = singles.tile([128, 32], bf16)
        nc.sync.dma_start(out=red8_sb[:], in_=red8)
        ones_sb = singles.tile([128, 1], fp32)
        nc.sync.dma_start(out=ones_sb[:], in_=ones128)
        lab_e_sb = singles.tile([128, E_COLS], bf16)
        nc.sync.dma_start(out=lab_e_sb[:], in_=lab_e)

        # ---------------- pass 1: load X (cast to bf16), transpose, cluster sums
        x_r = x_in.rearrange("d (j i) -> j d i", j=J)  # [J, D, M]
        xbf = []
        xt = []
        oht = []
        for t in range(nt):
            xb = xbf_pool.tile([128, 2048], bf16, tag="xbf")
            # partition p = 16*j + dd ; col i local to tile.  out is plain 2-D:
            # src iteration (j, d, i) matches dst (p, i) element order.
            nc.gpsimd.dma_start(
                out=xb[:],
                in_=x_r[:, :, 2048 * t : 2048 * (t + 1)],
            )
            xbf.append(xb)
            # contiguous [128,128] transpose blocks: out[f, p] = in[p, f]
            # with p = 16*j + dd, so chunk j occupies cols 16j..16j+16.
            xtt = xt_pool.tile([128, 16, 128], bf16, tag="xt")
            xt.append(xtt)
            oh = oht_pool.tile([128, 16, J, K], bf16, tag="oht")
            nc.sync.dma_start(out=oh[:], in_=oh_t[:, 16 * t : 16 * (t + 1), :, :])
            oht.append(oh)

        for t in range(nt):
            for cb in range(16):
                # transpose the 128-col block: out rows = points, col groups = (j, dd)
                nc.sync.dma_start_transpose(
                    out=xt[t][:, cb, :],
                    in_=xbf[t][:, 128 * cb : 128 * (cb + 1)],
                )

        # full-bank tile: the CoreSim PSUM pending-zero bookkeeping needs
        # row size == bank size when multiple column groups share a tile
        ps_sums = ps_sums_pool.tile([128, 512], fp32)
        cnt = 0
        for t in range(nt):
            for cb in range(16):
                for j in range(J):
                    g = cnt % 4
                    nc.tensor.matmul(
                        ps_sums[32 * g : 32 * g + K, 0:D],
                        oht[t][:, cb, j, :],
                        xt[t][:, cb, 16 * j : 16 * j + D],
                        start=(cnt < 4),
                        stop=(cnt >= nt * 16 * J - 4),
                        tile_position=(0, 32 * g),
                        skip_group_check=True,
                    )
                    cnt += 1

        # ---------------- centers (tiny ops)
        # TensorTensor may read at most one PSUM operand: copy slabs to SBUF.
        slabs = []
        for g in range(4):
            sl = tiny.tile([K, D], fp32, tag=f"slab{g}")
            nc.scalar.copy(out=sl[:], in_=ps_sums[32 * g : 32 * g + K, 0:D])
            slabs.append(sl)
        s01 = tiny.tile([K, D], fp32, tag="s01")
        nc.vector.tensor_add(s01[:], slabs[0][:], slabs[1][:])
        s23 = tiny.tile([K, D], fp32, tag="s23")
        nc.vector.tensor_add(s23[:], slabs[2][:], slabs[3][:])
        s_sb = tiny.tile([K, D + 1], fp32, tag="s_sb")
        nc.vector.tensor_add(s_sb[:, 0:D], s01[:], s23[:])
        # counts: per-partition masked counts of lab_e, then reduce over
        # partitions with a [128,8]-stationary x ones matmul -> [8, 1]
        cntp = tiny.tile([128, K], fp32, tag="cntp")
        cnt_scratch = singles.tile([128, E_COLS], bf16)
        for k in range(K):
            nc.vector.tensor_scalar(
                out=cnt_scratch[:],
                in0=lab_e_sb[:],
                scalar1=float(k),
                scalar2=None,
                op0=mybir.AluOpType.is_equal,
                op1=mybir.AluOpType.add,
                accum_out=cntp[:, k : k + 1],
            )
        ps_cnt = ps_sums_pool.tile([K, 1], fp32, tag="ps_cnt")
        nc.tensor.matmul(ps_cnt[:], cntp[:], ones_sb[:], start=True, stop=True)
        nc.scalar.copy(out=s_sb[:, D : D + 1], in_=ps_cnt[:])
        nc.sync.dma_start(out=out_stats, in_=s_sb[:])

        rec = tiny.tile([K, 1], fp32, tag="rec")
        nc.vector.reciprocal(rec[:], s_sb[:, D : D + 1])
        c_bf = tiny.tile([K, D], bf16, tag="c_bf")
        nc.vector.tensor_scalar(
            out=c_bf[:],
            in0=s_sb[:, 0:D],
            scalar1=rec[:],
            scalar2=None,
            op0=mybir.AluOpType.mult,
        )
        # W_cblk[(8j+k), (16j'+dd)] = delta(j,j') * c[k, dd]
        w_cblk = singles.tile([64, 128], bf16)
        nc.vector.memset(w_cblk[:], 0.0)
        for j in range(J):
            nc.sync.dma_start(
                out=w_cblk[8 * j : 8 * j + K, 16 * j : 16 * j + D], in_=c_bf[:]
            )

        # ---------------- pass 2
        e_dense = singles.tile([128, E_COLS], bf16)
        nc.vector.memset(e_dense[:], 0.0)
        for t in range(nt):
            ohjk_sb = p2.tile([64, 2048], bf16, tag="ohjk")
            nc.sync.dma_start(
                out=ohjk_sb[:], in_=oh_jk[:, 2048 * t : 2048 * (t + 1)]
            )
            c_ownb = p2.tile([128, 2048], bf16, tag="c_ownb")
            for b in range(4):
                ps_c = ps_c_pool.tile([128, 512], fp32)
                nc.tensor.matmul(
                    ps_c[:],
                    w_cblk[:],
                    ohjk_sb[:, 512 * b : 512 * (b + 1)],
                    start=True,
                    stop=True,
                )
                nc.scalar.copy(
                    out=c_ownb[:, 512 * b : 512 * (b + 1)], in_=ps_c[:]
                )
            dv = p2.tile([128, 2048], bf16, tag="dv")
            nc.vector.tensor_tensor(
                out=dv[:], in0=xbf[t][:], in1=c_ownb[:], op=mybir.AluOpType.subtract
            )
            sq = p2.tile([128, 2048], bf16, tag="sq")
            nc.vector.tensor_tensor(
                out=sq[:], in0=dv[:], in1=dv[:], op=mybir.AluOpType.mult
            )
            ps_e = ps_e_pool.tile([128, 512], fp32)
            for b in range(4):
                nc.tensor.matmul(
                    ps_e[32 * b : 32 * b + 32, :],
                    red8_sb[:],
                    sq[:, 512 * b : 512 * (b + 1)],
                    start=True,
                    stop=True,
                    tile_position=(0, 32 * b),
                    skip_group_check=True,
                )
            s_bf = p2b.tile([128, 512], bf16, tag="s_bf")
            nc.scalar.activation(
                out=s_bf[:], in_=ps_e[:], func=mybir.ActivationFunctionType.Sqrt
            )
            for v in range(4):
                # e_dense[16*t + j, 512*v + f] = s of point (chunk j,
                # pos 2048*t + 512*v + f) -- contiguous partition ranges.
                nc.sync.dma_start(
                    out=e_dense[8 * t : 8 * t + J, 512 * v : 512 * (v + 1)],
                    in_=s_bf[32 * v : 32 * v + J, :],
                )

        m_e = singles.tile([128, E_COLS], bf16)
        nc.vector.tensor_scalar(
            out=m_e[:],
            in0=e_dense[:],
            scalar1=-float(DELTA_VAR),
            scalar2=0.0,
            op0=mybir.AluOpType.add,
            op1=mybir.AluOpType.max,
        )
        msq = singles.tile([128, E_COLS], bf16)
        nc.vector.tensor_tensor(
            out=msq[:], in0=m_e[:], in1=m_e[:], op=mybir.AluOpType.mult
        )
        v_sb = tiny.tile([128, K], fp32, tag="v_sb")
        scratch = singles.tile([128, E_COLS], bf16)
        for k in range(K):
            nc.vector.scalar_tensor_tensor(
                out=scratch[:],
                in0=lab_e_sb[:],
                scalar=float(k),
                in1=msq[:],
                op0=mybir.AluOpType.is_equal,
                op1=mybir.AluOpType.mult,
                accum_out=v_sb[:, k : k + 1],
            )
        nc.sync.dma_start(out=out_var, in_=v_sb[:])

    _split_multi_waits(nc)
    return nc


# ---------------------------------------------------------------------------
# host-side input prep
def prep_core_inputs(x_c, labels_c, nt=16):
    """x_c fp32 [16, NPTS] (contiguous), labels_c int [NPTS] -> in_map."""
    M = 2048 * nt
    NPTS = J * M
    CB = M // 128
    bf = _bf16()
    lab = labels_c.astype(np.int64)

    l3 = lab.reshape(J, CB, 128)  # [j, cb, p]
    oh = (l3[..., None] == np.arange(K)).astype(bf)  # [j, cb, p, k]
    oh_t = np.ascontiguousarray(oh.transpose(2, 1, 0, 3))  # [128, cb, j, k]

    # oh_jk[8*j + k, f] = (labels[j*M + f] == k)
    oh_jk = (
        (lab.reshape(J, 1, M) == np.arange(K).reshape(1, K, 1))
        .reshape(64, M)
        .astype(bf)
    )

    # e_dense layout: partition 8*t + j (t < nt, j < 8),
    #                 col u -> point j*M + 2048*t + u
    lab_e = np.full((128, 2048), -1.0, dtype=np.float32)
    l4 = lab.reshape(J, nt, 2048)  # [j, t, u]
    for t in range(nt):
        lab_e[8 * t : 8 * t + J, :] = l4[:, t, :]
    lab_e = lab_e.astype(bf)

    red8 = np.zeros((128, 32), dtype=bf)
    for p in range(128):
        red8[p, p // 16] = 1.0
    ones128 = np.ones((128, 1), dtype=np.float32)

    return {
        "x": np.ascontiguousarray(x_c),
        "oh_t": oh_t,
        "oh_jk": oh_jk,
        "lab_e": lab_e,
        "red8": red8,
        "ones128": ones128,
    }


def finish_host(stats_list, var_list):
    """Combine per-core [K, D+1] sums|counts and [128, K] var partials."""
    losses = []
    for stats, vparts in zip(stats_list, var_list):
        S = stats[:, :D].astype(np.float64)
        m = stats[:, D].astype(np.float64)
        centers = S / m[:, None]
        V = vparts.astype(np.float64).sum(axis=0)  # [K]
        var_term = np.mean(V / m)
        dif = centers[None, :, :] - centers[:, None, :]
        dmat = np.sqrt((dif**2).sum(-1))
        dmat = dmat + np.eye(K) * DELTA_DIST
        dist_cost = np.clip(DELTA_DIST - dmat, 0.0, None) ** 2
        dist_term = dist_cost.sum() / (K * (K - 1))
        cn = np.sqrt((centers**2).sum(-1))
        reg_term = np.mean(np.clip(cn - np.sqrt(float(D)), 0.0, None) ** 2)
        losses.append(var_term + dist_term + reg_term)
    return np.float32(np.mean(losses))


# ---------------------------------------------------------------------------
_CACHE = {}


def _get_nc():
    if "nc" not in _CACHE:
        _CACHE["nc"] = build_nc(nt=16, num_devices=NCORES)
    return _CACHE["nc"]


def run_device(in_maps, trace=False):
    from concourse.bass_utils import run_bass_kernel_spmd

    if trace:
        install_ntff_hook()
    nc = _get_nc()
    return run_bass_kernel_spmd(
        nc, in_maps, core_ids=list(range(NCORES)), trace=trace
    )


def kernel(data, labels, n_clusters):
    assert int(n_clusters) == K
    assert data.shape == (B, D, HH, WW)
    x = np.asarray(data, dtype=np.float32).reshape(B, D, N)
    lab = np.asarray(labels).reshape(B, N)
    in_maps = [prep_core_inputs(x[c], lab[c]) for c in range(NCORES)]
    res = run_device(in_maps, trace=False)
    stats = [r["out_stats"] for r in res.results]
    vparts = [r["out_var"] for r in res.results]
    return finish_host(stats, vparts)



# revision 4
# speedup vs baseline: 3.8335x; 3.8335x over previous
"""Discriminative-loss kernel for Trainium2 (Bass/Tile), 8-core data-parallel.

One core per batch sample.  All label-derived tensors and all x re-layouts
(cast + transpose) are prepared on the host; the device streams each tensor
exactly once from HBM with large contiguous DMAs.

Device program per core (N = 262144 points, d = 16, K = 8):
  phase B  cluster sums:  256 "superblock" matmuls.  Stationary = x
           slab [128 pts, (b,dd)=128] fp8, moving = onehot [128 pts,
           (b,k)=64] fp8, all accumulated into one PSUM tile [128, 64].
           Diagonal blocks (b==b') hold per-slot cluster sums; off-diagonal
           cross terms are discarded.  Sum of 8 diagonal [16,8] blocks
           -> S[dd, k]; centers c = S * (1/m) (counts m from host labels).
  pass 2   per 2048-col tile of x_j (row = 16 j + dd):
           gather-mm (blockdiag-c stationary fp8, onehot moving fp8)
           -> c_own PSUM; DVE subtract (x - c_own) -> dv bf16; ACT square;
           4 col-tiled reduce matmuls (sum over dd) -> dist^2 PSUM;
           ACT sqrt -> e bf16; repack DMA -> e_dense [128, 2048].
  tail     relu(e-1)^2 on DVE, 8 masked accumulations -> V[128, K].
  host     centers / dist / reg terms + final mean from S, V, counts
           (O(K^2 d) flops on reduced stats only).
"""

import contextlib
import ctypes
import sys
import types

import numpy as np

# ---------------------------------------------------------------------------
# problem constants (hardcoded per contract)
B, D, HH, WW, K = 8, 16, 512, 512, 8
N = HH * WW            # 262144 points per sample
J = 8                  # chunk rows: x_j row = 16*j + dd
NCORES = 8
DELTA_VAR = 1.0
DELTA_DIST = 2.0
NCH = 4                # DMA chunks per big tensor

_ML = None


def _mld():
    global _ML
    if _ML is None:
        import ml_dtypes

        _ML = ml_dtypes
    return _ML


def _bf16():
    return np.dtype(_mld().bfloat16)


def _f8():
    return np.dtype(_mld().float8_e4m3)


# ---------------------------------------------------------------------------
# walrus workaround: this toolchain allows only ONE sync-wait per
# instruction; spread extras onto preceding same-engine nops.
def _split_multi_waits(nc):
    from concourse import mybir

    n = 0
    for f in nc.m.functions:
        for bb in f.blocks:
            new_insts = []
            for ins in bb.instructions:
                si = getattr(ins, "sync_info", None)
                waits = list(si.on_wait) if si is not None and si.on_wait else []
                if len(waits) > 1:
                    for w in waits[:-1]:
                        n += 1
                        new_insts.append(
                            mybir.InstNoOp(
                                name=f"I-waitsplit-{n}",
                                engine=ins.engine,
                                bass_nofuse=True,
                                sync_info=mybir.SyncInfo(on_wait=[w], on_update=[]),
                            )
                        )
                    si.on_wait = waits[-1:]
                new_insts.append(ins)
            bb.instructions[:] = new_insts


# ---------------------------------------------------------------------------
# NTFF profiling hook (axon): lets run_bass_kernel_spmd(trace=True) work in
# this container. Harmless if the .so lacks the symbols.
def install_ntff_hook():
    try:
        import antenv

        if "antenv.axon_hooks" in sys.modules:
            return
        so_path = "/opt/axon/libaxon_pjrt.so"
        lib = ctypes.CDLL(so_path)
        if not hasattr(lib, "axon_start_nrt_profile"):
            return
        lib.axon_start_nrt_profile.argtypes = [
            ctypes.POINTER(ctypes.c_int64),
            ctypes.c_size_t,
        ]
        lib.axon_start_nrt_profile.restype = ctypes.c_int64
        lib.axon_stop_nrt_profile.argtypes = [ctypes.c_char_p]
        lib.axon_stop_nrt_profile.restype = ctypes.c_int64

        @contextlib.contextmanager
        def _hook(output_dir, device_ids):
            import jax

            jax.devices()
            if device_ids:
                ids = (ctypes.c_int64 * len(device_ids))(*device_ids)
                rc = lib.axon_start_nrt_profile(ids, len(device_ids))
            else:
                rc = lib.axon_start_nrt_profile(None, 0)
            if rc != 0:
                raise RuntimeError(f"axon_start_nrt_profile rc={rc}")
            try:
                yield
            finally:
                n = lib.axon_stop_nrt_profile(str(output_dir).encode())
                print(f"ntff profile: {n} file(s) -> {output_dir}", file=sys.stderr)

        mod = types.ModuleType("antenv.axon_hooks")
        mod.get_axon_ntff_profile_hook = lambda: _hook
        mod.set_axon_ntff_profile_hook = lambda h: None
        sys.modules["antenv.axon_hooks"] = mod
        antenv.axon_hooks = mod
    except Exception:
        pass


# ---------------------------------------------------------------------------
def build_nc(nt=16, num_devices=NCORES):
    """nt = number of 2048-col tiles of the x_j layout (16 = full problem)."""
    import concourse.bass as bass
    import concourse.tile as tile
    from concourse import mybir

    assert nt % NCH == 0
    M = 2048 * nt          # points per chunk row (j)
    SB = 16 * nt           # superblocks of 1024 points (J*M/1024)
    TPC = nt // NCH        # tiles per DMA chunk
    MC = 2048 * TPC        # x_j cols per DMA chunk
    SBC = SB // NCH        # superblocks per DMA chunk

    fp32 = mybir.dt.float32
    bf16 = mybir.dt.bfloat16
    fp8 = mybir.dt.float8e4

    nc = bass.Bass(
        "TRN2", target_bir_lowering=False, debug=False, num_devices=num_devices
    )

    x_j = nc.dram_tensor("x_j", [128, M], bf16, kind="ExternalInput").ap()
    x_t = nc.dram_tensor("x_t", [128, SB * 128], fp8, kind="ExternalInput").ap()
    oh_sb = nc.dram_tensor("oh_sb", [128, SB * 64], fp8, kind="ExternalInput").ap()
    oh_jk = nc.dram_tensor("oh_jk", [64, M], fp8, kind="ExternalInput").ap()
    lab_e = nc.dram_tensor("lab_e", [128, 2048], bf16, kind="ExternalInput").ap()
    red8 = nc.dram_tensor("red8", [128, 32], bf16, kind="ExternalInput").ap()
    m_inv = nc.dram_tensor("m_inv", [K, 1], fp32, kind="ExternalInput").ap()
    id16 = nc.dram_tensor("id16", [D, D], fp32, kind="ExternalInput").ap()
    out_s = nc.dram_tensor("out_s", [D, K], fp32, kind="ExternalOutput").ap()
    out_var = nc.dram_tensor("out_var", [128, K], fp32, kind="ExternalOutput").ap()

    with tile.TileContext(nc) as tc, contextlib.ExitStack() as ctx:
        # ---------------- pools
        xt_pool = ctx.enter_context(tc.tile_pool(name="xt", bufs=NCH))
        ohsb_pool = ctx.enter_context(tc.tile_pool(name="ohsb", bufs=NCH))
        xj_pool = ctx.enter_context(tc.tile_pool(name="xj", bufs=NCH))
        ohjk_pool = ctx.enter_context(tc.tile_pool(name="ohjk", bufs=NCH))
        singles = ctx.enter_context(tc.tile_pool(name="singles", bufs=1))
        dv_pool = ctx.enter_context(tc.tile_pool(name="dv", bufs=2))
        sq_pool = ctx.enter_context(tc.tile_pool(name="sq", bufs=2))
        sbf_pool = ctx.enter_context(tc.tile_pool(name="sbf", bufs=4))
        ps_cl_pool = ctx.enter_context(
            tc.tile_pool(name="ps_cl", bufs=1, space="PSUM")
        )
        ps_c_pool = ctx.enter_context(tc.tile_pool(name="ps_c", bufs=2, space="PSUM"))
        ps_e_pool = ctx.enter_context(tc.tile_pool(name="ps_e", bufs=2, space="PSUM"))

        # ---------------- input DMAs (sync/SP ring drains in issue order:
        # phase-B data first, then pass-2 data)
        red8_sb = singles.tile([128, 32], bf16)
        nc.sync.dma_start(out=red8_sb[:], in_=red8)
        m_inv_sb = singles.tile([K, 1], fp32)
        nc.sync.dma_start(out=m_inv_sb[:], in_=m_inv)
        id16_sb = singles.tile([D, D], fp32)
        nc.sync.dma_start(out=id16_sb[:], in_=id16)

        xt = []
        ohsb = []
        for c in range(NCH):
            xtt = xt_pool.tile([128, SBC * 128], fp8, tag="xt")
            nc.sync.dma_start(out=xtt[:], in_=x_t[:, SBC * 128 * c : SBC * 128 * (c + 1)])
            xt.append(xtt)
            oht = ohsb_pool.tile([128, SBC * 64], fp8, tag="ohsb")
            nc.sync.dma_start(out=oht[:], in_=oh_sb[:, SBC * 64 * c : SBC * 64 * (c + 1)])
            ohsb.append(oht)

        xj = []
        ohjk = []
        lab_e_sb = None
        for c in range(NCH):
            xjt = xj_pool.tile([128, MC], bf16, tag="xj")
            nc.sync.dma_start(out=xjt[:], in_=x_j[:, MC * c : MC * (c + 1)])
            xj.append(xjt)
            ojt = ohjk_pool.tile([64, MC], fp8, tag="ohjk")
            nc.sync.dma_start(out=ojt[:], in_=oh_jk[:, MC * c : MC * (c + 1)])
            ohjk.append(ojt)
            if c == 1:
                lab_e_sb = singles.tile([128, 2048], bf16)
                nc.sync.dma_start(out=lab_e_sb[:], in_=lab_e)
        if lab_e_sb is None:  # small-nt builds
            lab_e_sb = singles.tile([128, 2048], bf16)
            nc.sync.dma_start(out=lab_e_sb[:], in_=lab_e)

        # ---------------- phase B: cluster sums
        ps_cl = ps_cl_pool.tile([128, 512], fp32, tag="cl")
        for c in range(NCH):
            for l in range(SBC):
                g = SBC * c + l
                nc.tensor.matmul(
                    ps_cl[:, 0:64],
                    xt[c][:, 128 * l : 128 * (l + 1)],
                    ohsb[c][:, 64 * l : 64 * (l + 1)],
                    start=(g == 0),
                    stop=(g == SB - 1),
                )

        # ---------------- centers
        # diag blocks: rows 16b..16b+16 x cols 8b..8b+8 hold sums of slot b
        cl_sb = singles.tile([128, 64], fp32)
        nc.scalar.copy(out=cl_sb[:], in_=ps_cl[:, 0:64])
        dstack = singles.tile([D, 8, K], fp32)
        for b in range(8):
            nc.scalar.dma_start(
                out=dstack[:, b, :], in_=cl_sb[16 * b : 16 * b + D, 8 * b : 8 * b + K]
            )
        h1 = singles.tile([D, 4, K], fp32)
        nc.vector.tensor_add(h1[:], dstack[:, 0:4, :], dstack[:, 4:8, :])
        h2 = singles.tile([D, 2, K], fp32)
        nc.vector.tensor_add(h2[:], h1[:, 0:2, :], h1[:, 2:4, :])
        acc = singles.tile([D, K], fp32)
        nc.vector.tensor_add(acc[:], h2[:, 0, :], h2[:, 1, :])
        nc.scalar.dma_start(out=out_s, in_=acc[:])

        # c[k, dd] = S[dd, k] / m_k  (PE transpose, then ACT scale-copy)
        ps_t = ps_cl_pool.tile([K, D], fp32, tag="pt")
        nc.tensor.transpose(ps_t[:], acc[:], id16_sb[:])
        c_f8 = singles.tile([K, D], fp8)
        nc.scalar.activation(
            out=c_f8[:],
            in_=ps_t[:],
            func=mybir.ActivationFunctionType.Copy,
            scale=m_inv_sb[:],
        )
        # W_cblk[(8j+k), (16j'+dd)] = delta(j,j') * c[k, dd]
        w_cblk = singles.tile([64, 128], fp8)
        nc.vector.memset(w_cblk[:], 0.0)
        for j in range(J):
            nc.scalar.dma_start(
                out=w_cblk[8 * j : 8 * j + K, 16 * j : 16 * j + D], in_=c_f8[:]
            )

        # ---------------- pass 2
        e_dense = singles.tile([128, 2048], bf16)
        for t in range(nt):
            c = t // TPC
            tb = 2048 * (t % TPC)
            dv = dv_pool.tile([128, 2048], bf16, tag="dv")
            for h in range(2):
                ps_c = ps_c_pool.tile([128, 1024], fp32)
                for q in range(2):
                    base = tb + 1024 * h + 512 * q
                    nc.tensor.matmul(
                        ps_c[:, 512 * q : 512 * (q + 1)],
                        w_cblk[:],
                        ohjk[c][:, base : base + 512],
                        start=True,
                        stop=True,
                    )
                nc.vector.tensor_tensor(
                    out=dv[:, 1024 * h : 1024 * (h + 1)],
                    in0=xj[c][:, tb + 1024 * h : tb + 1024 * (h + 1)],
                    in1=ps_c[:],
                    op=mybir.AluOpType.subtract,
                )
            sq = sq_pool.tile([128, 2048], bf16, tag="sq")
            nc.scalar.activation(
                out=sq[:], in_=dv[:], func=mybir.ActivationFunctionType.Square
            )
            ps_e = ps_e_pool.tile([128, 512], fp32)
            for v in range(4):
                nc.tensor.matmul(
                    ps_e[32 * v : 32 * v + 32, :],
                    red8_sb[:],
                    sq[:, 512 * v : 512 * (v + 1)],
                    start=True,
                    stop=True,
                    tile_position=(0, 32 * v),
                    skip_group_check=True,
                )
            sbf = sbf_pool.tile([128, 512], bf16, tag="sbf")
            nc.scalar.activation(
                out=sbf[:], in_=ps_e[:], func=mybir.ActivationFunctionType.Sqrt
            )
            for v in range(4):
                nc.scalar.dma_start(
                    out=e_dense[8 * t : 8 * t + J, 512 * v : 512 * (v + 1)],
                    in_=sbf[32 * v : 32 * v + J, :],
                )

        # ---------------- tail: V[k] = sum_{L=k} relu(e - 1)^2
        m_e = singles.tile([128, 2048], bf16)
        nc.vector.tensor_scalar(
            out=m_e[:],
            in0=e_dense[:],
            scalar1=-float(DELTA_VAR),
            scalar2=0.0,
            op0=mybir.AluOpType.add,
            op1=mybir.AluOpType.max,
        )
        msq = singles.tile([128, 2048], bf16)
        nc.vector.tensor_tensor(
            out=msq[:], in0=m_e[:], in1=m_e[:], op=mybir.AluOpType.mult
        )
        v_sb = singles.tile([128, K], fp32)
        scratch = singles.tile([128, 2048], bf16)
        for k in range(K):
            nc.vector.scalar_tensor_tensor(
                out=scratch[:],
                in0=lab_e_sb[:],
                scalar=float(k),
                in1=msq[:],
                op0=mybir.AluOpType.is_equal,
                op1=mybir.AluOpType.mult,
                accum_out=v_sb[:, k : k + 1],
            )
        nc.scalar.dma_start(out=out_var, in_=v_sb[:])

    _split_multi_waits(nc)
    return nc


# ---------------------------------------------------------------------------
# host-side input prep
def prep_core_inputs(x_c, labels_c, nt=16):
    """x_c fp32 [16, NPTS] (d-major), labels_c int [NPTS] -> in_map."""
    M = 2048 * nt
    SB = 16 * nt
    NPTS = J * M
    bf = _bf16()
    f8 = _f8()
    x = np.ascontiguousarray(x_c, dtype=np.float32)
    lab = labels_c.astype(np.int64)
    assert x.shape == (D, NPTS) and lab.shape == (NPTS,)

    # x_j[16j+dd, i] = x[dd, j*M+i]
    x_j = np.ascontiguousarray(
        x.reshape(D, J, M).transpose(1, 0, 2).reshape(J * D, M)
    ).astype(bf)
    # x_t[nn, 128s+16b+dd] = x[dd, 1024s+128b+nn]
    x_t = np.ascontiguousarray(
        x.reshape(D, SB, 8, 128).transpose(3, 1, 2, 0).reshape(128, SB * 128)
    ).astype(f8)
    # oh_sb[nn, 64s+8b+k] = (lab[1024s+128b+nn] == k)
    l_sb = lab.reshape(SB, 8, 128)
    oh_sb = np.ascontiguousarray(
        (l_sb[:, :, :, None] == np.arange(K)).transpose(2, 0, 1, 3).reshape(128, SB * K * 8)
    ).astype(f8)
    # oh_jk[8j+k, i] = (lab[j*M+i] == k)
    oh_jk = np.ascontiguousarray(
        (lab.reshape(J, 1, M) == np.arange(K).reshape(1, K, 1)).reshape(J * K, M)
    ).astype(f8)
    # lab_e[8t+j, u] = lab[j*M + 2048t + u]
    lab_e = np.full((128, 2048), -1.0, dtype=np.float32)
    lab_e[: 8 * nt, :] = lab.reshape(J, nt, 2048).transpose(1, 0, 2).reshape(8 * nt, 2048)
    lab_e = lab_e.astype(bf)

    red8 = np.zeros((128, 32), dtype=bf)
    for p in range(128):
        red8[p, p // 16] = 1.0
    m = np.bincount(lab, minlength=K).astype(np.float64)
    m_inv = (1.0 / np.maximum(m, 1.0)).astype(np.float32).reshape(K, 1)
    id16 = np.eye(D, dtype=np.float32)

    return {
        "x_j": x_j,
        "x_t": x_t,
        "oh_sb": oh_sb,
        "oh_jk": oh_jk,
        "lab_e": lab_e,
        "red8": red8,
        "m_inv": m_inv,
        "id16": id16,
    }


def finish_host(s_list, var_list, counts_list):
    """Combine per-core S [D, K] sums, V [128, K] partials, counts [K]."""
    losses = []
    for S_dk, vparts, m in zip(s_list, var_list, counts_list):
        S = S_dk.astype(np.float64).T          # [K, D]
        m = m.astype(np.float64)
        centers = S / m[:, None]
        V = vparts.astype(np.float64).sum(axis=0)  # [K]
        var_term = np.mean(V / m)
        dif = centers[None, :, :] - centers[:, None, :]
        dmat = np.sqrt((dif**2).sum(-1)) + np.eye(K) * DELTA_DIST
        dist_cost = np.clip(DELTA_DIST - dmat, 0.0, None) ** 2
        dist_term = dist_cost.sum() / (K * (K - 1))
        cn = np.sqrt((centers**2).sum(-1))
        reg_term = np.mean(np.clip(cn - np.sqrt(float(D)), 0.0, None) ** 2)
        losses.append(var_term + dist_term + reg_term)
    return np.float32(np.mean(losses))


# ---------------------------------------------------------------------------
_CACHE = {}


def _get_nc():
    if "nc" not in _CACHE:
        _CACHE["nc"] = build_nc(nt=16, num_devices=NCORES)
    return _CACHE["nc"]


def run_device(in_maps, trace=False):
    from concourse.bass_utils import run_bass_kernel_spmd

    if trace:
        install_ntff_hook()
    nc = _get_nc()
    return run_bass_kernel_spmd(
        nc, in_maps, core_ids=list(range(NCORES)), trace=trace
    )


def kernel(data, labels, n_clusters):
    assert int(n_clusters) == K
    assert data.shape == (B, D, HH, WW)
    x = np.asarray(data, dtype=np.float32).reshape(B, D, N)
    lab = np.asarray(labels).reshape(B, N)
    in_maps = [prep_core_inputs(x[c], lab[c]) for c in range(NCORES)]
    counts = [np.bincount(lab[c], minlength=K) for c in range(NCORES)]
    res = run_device(in_maps, trace=False)
    return finish_host(
        [r["out_s"] for r in res.results],
        [r["out_var"] for r in res.results],
        counts,
    )


# revision 13
# speedup vs baseline: 4.3567x; 1.1365x over previous
"""Discriminative-loss kernel for Trainium2 (Bass/Tile), 8-core data-parallel.

One core per batch sample.  All label-derived tensors and all x re-layouts
(cast + transpose) are prepared on the host; the device streams each tensor
exactly once from HBM with large contiguous DMAs.

Device program per core (N = 262144 points, d = 16, K = 8):
  phase B  cluster sums:  256 "superblock" matmuls.  Stationary = x
           slab [128 pts, (b,dd)=128] fp8, moving = onehot [128 pts,
           (b,k)=64] fp8, all accumulated into one PSUM tile [128, 64].
           Diagonal blocks (b==b') hold per-slot cluster sums; off-diagonal
           cross terms are discarded.  Sum of 8 diagonal [16,8] blocks
           -> S[dd, k]; centers c = S * (1/m) (counts m from host labels).
  pass 2   per 2048-col tile of x_j (row = 16 j + dd):
           gather-mm (blockdiag-c stationary fp8, onehot moving fp8)
           -> c_own PSUM; DVE subtract (x - c_own) -> dv bf16; ACT square;
           4 col-tiled reduce matmuls (sum over dd) -> dist^2 PSUM;
           ACT sqrt -> e bf16; repack DMA -> e_dense [128, 2048].
  tail     relu(e-1)^2 on DVE, 8 masked accumulations -> V[128, K].
  host     centers / dist / reg terms + final mean from S, V, counts
           (O(K^2 d) flops on reduced stats only).
"""

import contextlib
import ctypes
import sys
import types

import numpy as np

# ---------------------------------------------------------------------------
# problem constants (hardcoded per contract)
B, D, HH, WW, K = 8, 16, 512, 512, 8
N = HH * WW            # 262144 points per sample
J = 8                  # chunk rows: x_j row = 16*j + dd
NCORES = 8
DELTA_VAR = 1.0
DELTA_DIST = 2.0
NCH = 4                # DMA chunks per big tensor

_ML = None


def _mld():
    global _ML
    if _ML is None:
        import ml_dtypes

        _ML = ml_dtypes
    return _ML


def _bf16():
    return np.dtype(_mld().bfloat16)


def _f8():
    return np.dtype(_mld().float8_e4m3)


# ---------------------------------------------------------------------------
# walrus workaround: this toolchain allows only ONE sync-wait per
# instruction; spread extras onto preceding same-engine nops.
def _split_multi_waits(nc):
    from concourse import mybir

    n = 0
    for f in nc.m.functions:
        for bb in f.blocks:
            new_insts = []
            for ins in bb.instructions:
                si = getattr(ins, "sync_info", None)
                waits = list(si.on_wait) if si is not None and si.on_wait else []
                if len(waits) > 1:
                    for w in waits[:-1]:
                        n += 1
                        new_insts.append(
                            mybir.InstNoOp(
                                name=f"I-waitsplit-{n}",
                                engine=ins.engine,
                                bass_nofuse=True,
                                sync_info=mybir.SyncInfo(on_wait=[w], on_update=[]),
                            )
                        )
                    si.on_wait = waits[-1:]
                new_insts.append(ins)
            bb.instructions[:] = new_insts


# ---------------------------------------------------------------------------
# NTFF profiling hook (axon): lets run_bass_kernel_spmd(trace=True) work in
# this container. Harmless if the .so lacks the symbols.
def install_ntff_hook():
    try:
        import antenv

        if "antenv.axon_hooks" in sys.modules:
            return
        so_path = "/opt/axon/libaxon_pjrt.so"
        lib = ctypes.CDLL(so_path)
        if not hasattr(lib, "axon_start_nrt_profile"):
            return
        lib.axon_start_nrt_profile.argtypes = [
            ctypes.POINTER(ctypes.c_int64),
            ctypes.c_size_t,
        ]
        lib.axon_start_nrt_profile.restype = ctypes.c_int64
        lib.axon_stop_nrt_profile.argtypes = [ctypes.c_char_p]
        lib.axon_stop_nrt_profile.restype = ctypes.c_int64

        @contextlib.contextmanager
        def _hook(output_dir, device_ids):
            import jax

            jax.devices()
            if device_ids:
                ids = (ctypes.c_int64 * len(device_ids))(*device_ids)
                rc = lib.axon_start_nrt_profile(ids, len(device_ids))
            else:
                rc = lib.axon_start_nrt_profile(None, 0)
            if rc != 0:
                raise RuntimeError(f"axon_start_nrt_profile rc={rc}")
            try:
                yield
            finally:
                n = lib.axon_stop_nrt_profile(str(output_dir).encode())
                print(f"ntff profile: {n} file(s) -> {output_dir}", file=sys.stderr)

        mod = types.ModuleType("antenv.axon_hooks")
        mod.get_axon_ntff_profile_hook = lambda: _hook
        mod.set_axon_ntff_profile_hook = lambda h: None
        sys.modules["antenv.axon_hooks"] = mod
        antenv.axon_hooks = mod
    except Exception:
        pass


# ---------------------------------------------------------------------------
def build_nc(nt=16, num_devices=NCORES):
    """nt = number of 2048-col tiles of the x_j layout (16 = full problem)."""
    import concourse.bass as bass
    import concourse.tile as tile
    from concourse import mybir

    assert nt % NCH == 0
    M = 2048 * nt          # points per chunk row (j)
    SB = 16 * nt           # superblocks of 1024 points (J*M/1024)
    TPC = nt // NCH        # tiles per DMA chunk
    MC = 2048 * TPC        # x_j cols per DMA chunk
    SBC = SB // NCH        # superblocks per DMA chunk

    fp32 = mybir.dt.float32
    bf16 = mybir.dt.bfloat16
    fp8 = mybir.dt.float8e4

    nc = bass.Bass(
        "TRN2", target_bir_lowering=False, debug=False, num_devices=num_devices
    )

    x_j = nc.dram_tensor("x_j", [128, M], bf16, kind="ExternalInput").ap()
    x_t = nc.dram_tensor("x_t", [128, SB * 128], fp8, kind="ExternalInput").ap()
    oh_sb = nc.dram_tensor("oh_sb", [128, SB * 64], fp8, kind="ExternalInput").ap()
    oh_jk = nc.dram_tensor("oh_jk", [64, M], fp8, kind="ExternalInput").ap()
    lab_e = nc.dram_tensor("lab_e", [128, 2048], bf16, kind="ExternalInput").ap()
    red8 = nc.dram_tensor("red8", [128, 32], bf16, kind="ExternalInput").ap()
    m_inv = nc.dram_tensor("m_inv", [K, 1], fp32, kind="ExternalInput").ap()
    out_s = nc.dram_tensor("out_s", [K, D], fp32, kind="ExternalOutput").ap()
    out_var = nc.dram_tensor("out_var", [128, K], fp32, kind="ExternalOutput").ap()

    with tile.TileContext(nc) as tc, contextlib.ExitStack() as ctx:
        # ---------------- pools
        xt_pool = ctx.enter_context(tc.tile_pool(name="xt", bufs=NCH))
        ohsb_pool = ctx.enter_context(tc.tile_pool(name="ohsb", bufs=NCH))
        xj_pool = ctx.enter_context(tc.tile_pool(name="xj", bufs=NCH))
        ohjk_pool = ctx.enter_context(tc.tile_pool(name="ohjk", bufs=NCH))
        singles = ctx.enter_context(tc.tile_pool(name="singles", bufs=1))
        dv_pool = ctx.enter_context(tc.tile_pool(name="dv", bufs=2))
        sq_pool = ctx.enter_context(tc.tile_pool(name="sq", bufs=2))
        sbf_pool = ctx.enter_context(tc.tile_pool(name="sbf", bufs=8))
        ps_cl_pool = ctx.enter_context(
            tc.tile_pool(name="ps_cl", bufs=1, space="PSUM")
        )
        ps_c_pool = ctx.enter_context(tc.tile_pool(name="ps_c", bufs=2, space="PSUM"))
        ps_e_pool = ctx.enter_context(tc.tile_pool(name="ps_e", bufs=2, space="PSUM"))

        # ---------------- input DMAs (sync/SP ring drains in issue order:
        # phase-B data first, then pass-2 data)
        red8_sb = singles.tile([128, 32], bf16)
        nc.sync.dma_start(out=red8_sb[:], in_=red8)
        m_inv_sb = singles.tile([K, 1], fp32)
        nc.sync.dma_start(out=m_inv_sb[:], in_=m_inv)

        xt = []
        ohsb = []
        for c in range(NCH):
            xtt = xt_pool.tile([128, SBC * 128], fp8, tag="xt")
            nc.sync.dma_start(out=xtt[:], in_=x_t[:, SBC * 128 * c : SBC * 128 * (c + 1)])
            xt.append(xtt)
            oht = ohsb_pool.tile([128, SBC * 64], fp8, tag="ohsb")
            nc.sync.dma_start(out=oht[:], in_=oh_sb[:, SBC * 64 * c : SBC * 64 * (c + 1)])
            ohsb.append(oht)

        xj = []
        ohjk = []
        lab_e_sb = None
        for c in range(NCH):
            xjt = xj_pool.tile([128, MC], bf16, tag="xj")
            nc.sync.dma_start(out=xjt[:], in_=x_j[:, MC * c : MC * (c + 1)])
            xj.append(xjt)
            ojt = ohjk_pool.tile([64, MC], fp8, tag="ohjk")
            nc.sync.dma_start(out=ojt[:], in_=oh_jk[:, MC * c : MC * (c + 1)])
            ohjk.append(ojt)
            if c == 1:
                lab_e_sb = singles.tile([128, 2048], bf16)
                nc.sync.dma_start(out=lab_e_sb[:], in_=lab_e)
        if lab_e_sb is None:  # small-nt builds
            lab_e_sb = singles.tile([128, 2048], bf16)
            nc.sync.dma_start(out=lab_e_sb[:], in_=lab_e)

        # ---------------- phase B: cluster sums
        # out[(b,k), (b',dd)] accumulated over superblocks; diagonal b==b'
        # blocks hold per-slot cluster sums, directly [k, dd]-oriented.
        ps_cl = ps_cl_pool.tile([128, 512], fp32, tag="cl")
        for c in range(NCH):
            for l in range(SBC):
                g = SBC * c + l
                nc.tensor.matmul(
                    ps_cl[0:64, 0:128],
                    ohsb[c][:, 64 * l : 64 * (l + 1)],
                    xt[c][:, 128 * l : 128 * (l + 1)],
                    start=(g == 0),
                    stop=(g == SB - 1),
                )

        # ---------------- centers
        # diag blocks: rows 8b..8b+8 x cols 16b..16b+16 hold sums of slot b
        cl_sb = singles.tile([64, 128], fp32)
        nc.scalar.copy(out=cl_sb[:], in_=ps_cl[0:64, 0:128])
        dstack = singles.tile([K, 8, D], fp32)
        for b in range(8):
            nc.scalar.dma_start(
                out=dstack[:, b, :], in_=cl_sb[8 * b : 8 * b + K, 16 * b : 16 * b + D]
            )
        h1 = singles.tile([K, 4, D], fp32)
        nc.vector.tensor_add(h1[:], dstack[:, 0:4, :], dstack[:, 4:8, :])
        h2 = singles.tile([K, 2, D], fp32)
        nc.vector.tensor_add(h2[:], h1[:, 0:2, :], h1[:, 2:4, :])
        s_kd = singles.tile([K, D], fp32)
        nc.vector.tensor_add(s_kd[:], h2[:, 0, :], h2[:, 1, :])
        nc.sync.dma_start(out=out_s, in_=s_kd[:])

        # c[k, dd] = S[k, dd] / m_k
        c_f8 = singles.tile([K, D], fp8)
        nc.scalar.activation(
            out=c_f8[:],
            in_=s_kd[:],
            func=mybir.ActivationFunctionType.Copy,
            scale=m_inv_sb[:],
        )
        # W_cblk[(8j+k), (16j'+dd)] = delta(j,j') * c[k, dd]
        w_cblk = singles.tile([64, 128], fp8)
        nc.vector.memset(w_cblk[:], 0.0)
        for j in range(J):
            nc.scalar.dma_start(
                out=w_cblk[8 * j : 8 * j + K, 16 * j : 16 * j + D], in_=c_f8[:]
            )

        # ---------------- pass 2 (tile pairs: batched ACT squares)
        e_dense = singles.tile([128, 2048], bf16)
        for tp in range(nt // 2):
            dv = dv_pool.tile([128, 4096], bf16, tag="dv")
            for th in range(4):  # half-tiles of the pair
                t = 2 * tp + th // 2
                c = t // TPC
                base = 2048 * (t % TPC) + 1024 * (th % 2)
                ps_c = ps_c_pool.tile([128, 1024], fp32)
                for q in range(2):
                    nc.tensor.matmul(
                        ps_c[:, 512 * q : 512 * (q + 1)],
                        w_cblk[:],
                        ohjk[c][:, base + 512 * q : base + 512 * (q + 1)],
                        start=True,
                        stop=True,
                    )
                nc.vector.tensor_tensor(
                    out=dv[:, 1024 * th : 1024 * (th + 1)],
                    in0=xj[c][:, base : base + 1024],
                    in1=ps_c[:],
                    op=mybir.AluOpType.subtract,
                )
            sq = sq_pool.tile([128, 4096], bf16, tag="sq")
            nc.scalar.activation(
                out=sq[:], in_=dv[:], func=mybir.ActivationFunctionType.Square
            )
            for t in (2 * tp, 2 * tp + 1):
                sqb = 2048 * (t - 2 * tp)
                ps_e = ps_e_pool.tile([128, 512], fp32)
                for v in range(4):
                    nc.tensor.matmul(
                        ps_e[32 * v : 32 * v + 32, :],
                        red8_sb[:],
                        sq[:, sqb + 512 * v : sqb + 512 * (v + 1)],
                        start=True,
                        stop=True,
                        tile_position=(0, 32 * v),
                        skip_group_check=True,
                    )
                sbf = sbf_pool.tile([128, 512], bf16, tag="sbf")
                nc.scalar.activation(
                    out=sbf[:], in_=ps_e[:], func=mybir.ActivationFunctionType.Sqrt
                )
                for v in range(4):
                    nc.sync.dma_start(
                        out=e_dense[8 * t : 8 * t + J, 512 * v : 512 * (v + 1)],
                        in_=sbf[32 * v : 32 * v + J, :],
                    )

        # ---------------- tail: V[k] = sum_{L=k} relu(e - 1)^2
        m_e = singles.tile([128, 2048], bf16)
        nc.vector.tensor_scalar(
            out=m_e[:],
            in0=e_dense[:],
            scalar1=-float(DELTA_VAR),
            scalar2=0.0,
            op0=mybir.AluOpType.add,
            op1=mybir.AluOpType.max,
        )
        msq = singles.tile([128, 2048], bf16)
        nc.vector.tensor_tensor(
            out=msq[:], in0=m_e[:], in1=m_e[:], op=mybir.AluOpType.mult
        )
        v_sb = singles.tile([128, K], fp32)
        scratch = singles.tile([128, 2048], bf16)
        for k in range(K):
            nc.vector.scalar_tensor_tensor(
                out=scratch[:],
                in0=lab_e_sb[:],
                scalar=float(k),
                in1=msq[:],
                op0=mybir.AluOpType.is_equal,
                op1=mybir.AluOpType.mult,
                accum_out=v_sb[:, k : k + 1],
            )
        nc.sync.dma_start(out=out_var, in_=v_sb[:])

    _split_multi_waits(nc)
    return nc


# ---------------------------------------------------------------------------
# host-side input prep
def prep_core_inputs(x_c, labels_c, nt=16):
    """x_c fp32 [16, NPTS] (d-major), labels_c int [NPTS] -> in_map."""
    M = 2048 * nt
    SB = 16 * nt
    NPTS = J * M
    bf = _bf16()
    f8 = _f8()
    x = np.ascontiguousarray(x_c, dtype=np.float32)
    lab = labels_c.astype(np.int64)
    assert x.shape == (D, NPTS) and lab.shape == (NPTS,)

    # x_j[16j+dd, i] = x[dd, j*M+i]
    x_j = np.ascontiguousarray(
        x.reshape(D, J, M).transpose(1, 0, 2).reshape(J * D, M)
    ).astype(bf)
    # x_t[nn, 128s+16b+dd] = x[dd, 1024s+128b+nn]
    x_t = np.ascontiguousarray(
        x.reshape(D, SB, 8, 128).transpose(3, 1, 2, 0).reshape(128, SB * 128)
    ).astype(f8)
    # oh_sb[nn, 64s+8b+k] = (lab[1024s+128b+nn] == k)
    l_sb = lab.reshape(SB, 8, 128)
    oh_sb = np.ascontiguousarray(
        (l_sb[:, :, :, None] == np.arange(K)).transpose(2, 0, 1, 3).reshape(128, SB * K * 8)
    ).astype(f8)
    # oh_jk[8j+k, i] = (lab[j*M+i] == k)
    oh_jk = np.ascontiguousarray(
        (lab.reshape(J, 1, M) == np.arange(K).reshape(1, K, 1)).reshape(J * K, M)
    ).astype(f8)
    # lab_e[8t+j, u] = lab[j*M + 2048t + u]
    lab_e = np.full((128, 2048), -1.0, dtype=np.float32)
    lab_e[: 8 * nt, :] = lab.reshape(J, nt, 2048).transpose(1, 0, 2).reshape(8 * nt, 2048)
    lab_e = lab_e.astype(bf)

    red8 = np.zeros((128, 32), dtype=bf)
    for p in range(128):
        red8[p, p // 16] = 1.0
    m = np.bincount(lab, minlength=K).astype(np.float64)
    m_inv = (1.0 / np.maximum(m, 1.0)).astype(np.float32).reshape(K, 1)

    return {
        "x_j": x_j,
        "x_t": x_t,
        "oh_sb": oh_sb,
        "oh_jk": oh_jk,
        "lab_e": lab_e,
        "red8": red8,
        "m_inv": m_inv,
    }


def finish_host(s_list, var_list, counts_list):
    """Combine per-core S [K, D] sums, V [128, K] partials, counts [K]."""
    losses = []
    for S_kd, vparts, m in zip(s_list, var_list, counts_list):
        S = S_kd.astype(np.float64)            # [K, D]
        m = m.astype(np.float64)
        centers = S / m[:, None]
        V = vparts.astype(np.float64).sum(axis=0)  # [K]
        var_term = np.mean(V / m)
        dif = centers[None, :, :] - centers[:, None, :]
        dmat = np.sqrt((dif**2).sum(-1)) + np.eye(K) * DELTA_DIST
        dist_cost = np.clip(DELTA_DIST - dmat, 0.0, None) ** 2
        dist_term = dist_cost.sum() / (K * (K - 1))
        cn = np.sqrt((centers**2).sum(-1))
        reg_term = np.mean(np.clip(cn - np.sqrt(float(D)), 0.0, None) ** 2)
        losses.append(var_term + dist_term + reg_term)
    return np.float32(np.mean(losses))


# ---------------------------------------------------------------------------
_CACHE = {}


def _get_nc():
    if "nc" not in _CACHE:
        _CACHE["nc"] = build_nc(nt=16, num_devices=NCORES)
    return _CACHE["nc"]


def run_device(in_maps, trace=False):
    from concourse.bass_utils import run_bass_kernel_spmd

    if trace:
        install_ntff_hook()
    nc = _get_nc()
    return run_bass_kernel_spmd(
        nc, in_maps, core_ids=list(range(NCORES)), trace=trace
    )


def kernel(data, labels, n_clusters):
    assert int(n_clusters) == K
    assert data.shape == (B, D, HH, WW)
    x = np.asarray(data, dtype=np.float32).reshape(B, D, N)
    lab = np.asarray(labels).reshape(B, N)
    in_maps = [prep_core_inputs(x[c], lab[c]) for c in range(NCORES)]
    counts = [np.bincount(lab[c], minlength=K) for c in range(NCORES)]
    res = run_device(in_maps, trace=False)
    return finish_host(
        [r["out_s"] for r in res.results],
        [r["out_var"] for r in res.results],
        counts,
    )
